# revision 33
# baseline (speedup 1.0000x reference)
"""Trainium2 Bass kernel for nn_DiffusionGraphConv (gnn_message_passing).

Reference computation (B=64, N=1024, D=128=64+64, O=128, 2 supports,
2 diffusion steps):
    x0 = concat(inputs, state)                      # [B, N, D]
    y1 = S0 x0 ; z2 = S0 y1 ; y3 = S1 y1 ; z4 = S1 y3
    xs = [x0, y1, 2 z2 - x0, y3, 2 z4 - y1]
    out = concat_d(xs) @ W + bias                   # [B*N, O]

Algebraic refactor (host folds the +-/2x into the weight blocks, and the
feature projection commutes with the node-space supports):
    Wa = W0 - W2, Wb = W1 - W4, Wc = 2 W2, Wd = W3, We = 2 W4
    out = x0 Wa + S0 (x0 Wb + y1 Wc) + S1 (y1 Wd + y3 We)

Sharding: data-parallel over batch, 8 batches per NeuronCore, supports and
weights replicated. Per-core schedule (f32 PSUM accumulation, bf16 MMs):
    pass1: y1   = S0 x0          F-layout [n, (b,d)], f-paired FD512 MMs
    trans: y1T  via DMA XBAR     (off the PE; dedicated [128,128] dst tiles)
    pass2: y3T  = (S1 y1)^T      f-paired FD512 bf16 MMs
    feat:  P1 = x0 Wb + y1 Wc ; P2 = y1 Wd + y3 We   (bundled [Wc|Wd]
           moving operand; one strided copy into the merged pf tile)
    final: out = x0 Wa + S0 P1 + S1 P2 + bias        (FD512 MMs + regions)
Per-rep reloaded tensors (xa) do not alias late-phase consumers, so the
next rep's input DMA overlaps the current rep's compute.
"""
import sys

if "/opt/trn_rl_repo" not in sys.path:
    sys.path.insert(0, "/opt/trn_rl_repo")

import numpy as np
import ml_dtypes

import concourse.bass as bass
import concourse.mybir as mybir
from concourse import bacc, tile
from concourse.bass_utils import run_bass_kernel_spmd
from concourse.masks import make_identity

N_CORES = 8
B = 64
BL = B // N_CORES          # local batches per core
N = 1024                   # nodes
D = 128                    # input_size (64 input + 64 hidden)
O = 128                    # output_size
NT = N // 128              # node partition tiles
F32 = mybir.dt.float32
BF16 = mybir.dt.bfloat16
FP8 = mybir.dt.float8e4
DR = mybir.MatmulPerfMode.DoubleRow
NP_FP8 = ml_dtypes.float8_e4m3
NP_BF16 = ml_dtypes.bfloat16

_CACHE = {}


def _build(reps=1):
    nc = bacc.Bacc("TRN2", target_bir_lowering=False, debug=False,
                   num_devices=N_CORES)
    s0t_d = nc.dram_tensor("s0t", [N, N], BF16, kind="ExternalInput").ap()
    s1t_d = nc.dram_tensor("s1t", [N, N], BF16, kind="ExternalInput").ap()
    x0f_d = nc.dram_tensor("x0f", [N, BL * D], BF16, kind="ExternalInput").ap()
    x0t_d = nc.dram_tensor("x0t", [BL * D, N], BF16, kind="ExternalInput").ap()
    # weights side by side: [Wa | Wb | Wc | Wd | We]  ([Wc|Wd] is one
    # contiguous 256-wide moving operand for the bundled y1 projection)
    wf_d = nc.dram_tensor("wf", [D, 5 * O], BF16, kind="ExternalInput").ap()
    bias_d = nc.dram_tensor("biasb", [128, 512], F32, kind="ExternalInput").ap()
    out_d = nc.dram_tensor("out", [N, BL, O], BF16, kind="ExternalOutput").ap()

    with tile.TileContext(nc) as tc:
        with (
            tc.tile_pool(name="main", bufs=1) as mp,
            tc.tile_pool(name="outp", bufs=4) as op,
            tc.tile_pool(name="psb", bufs=5, space="PSUM") as pb,
            tc.tile_pool(name="pss", bufs=3, space="PSUM") as psm,
        ):
            # ---- persistent SBUF residents (DMA in consumption order) ----
            identf = mp.tile([128, 128], F32, tag="idf")
            make_identity(nc, identf[:])
            identr = mp.tile([128, 128], BF16, tag="idr")
            nc.vector.tensor_copy(identr[:], identf[:])
            s0t = []
            xa0 = []
            for j in range(NT):
                t = mp.tile([128, N], BF16, tag=f"s0t{j}", name=f"s0t{j}")
                nc.sync.dma_start(out=t[:], in_=s0t_d[j * 128:(j + 1) * 128, :])
                s0t.append(t)
                t = mp.tile([128, BL * D], BF16, tag=f"xa{j}", name=f"xa{j}_p",
                            bufs=2)
                nc.sync.dma_start(out=t[:], in_=x0f_d[j * 128:(j + 1) * 128, :])
                xa0.append(t)
            s1t = []
            for j in range(NT):
                t = mp.tile([128, N], BF16, tag=f"s1t{j}", name=f"s1t{j}")
                nc.sync.dma_start(out=t[:], in_=s1t_d[j * 128:(j + 1) * 128, :])
                s1t.append(t)
            x0t = []
            for b in range(BL):
                t = mp.tile([128, N], BF16, tag=f"x0t{b}", name=f"x0t{b}")
                nc.sync.dma_start(out=t[:], in_=x0t_d[b * 128:(b + 1) * 128, :])
                x0t.append(t)
            w_t = mp.tile([128, 5 * O], BF16, tag="wt")
            nc.sync.dma_start(out=w_t[:], in_=wf_d[:])
            bias_t = mp.tile([128, 512], F32, tag="bias")
            nc.sync.dma_start(out=bias_t[:], in_=bias_d[:])

            ci = 0

            def pcopy(dst, src):
                # alternate DVE / ACT for PSUM->SBUF moves
                nonlocal ci
                if ci % 2 == 0:
                    nc.vector.tensor_copy(dst, src)
                else:
                    nc.scalar.copy(dst, src)
                ci += 1

            def pscale(dst, src, mul):
                nonlocal ci
                if ci % 2 == 0:
                    nc.vector.tensor_scalar_mul(dst, src, mul)
                else:
                    nc.scalar.mul(dst, src, mul)
                ci += 1

            xa = xa0
            for rep in range(reps):
                # per-rep buffers
                yb16 = [mp.tile([128, BL, D], BF16, tag=f"yb{it}",
                                name=f"yb{it}_{rep}") for it in range(NT)]
                y1tt = [[mp.tile([128, 128], BF16, tag=f"y1t{b}_{nt}",
                                 name=f"y1t{b}_{nt}_{rep}")
                         for nt in range(NT)] for b in range(BL)]
                y3t = [mp.tile([128, N], BF16, tag=f"y3t{b}",
                               name=f"y3t{b}_{rep}") for b in range(BL)]
                # merged projections: pf[nt][:, 0, :] = P1, [:, 1, :] = P2
                pf = [mp.tile([128, 2, BL * O], BF16, tag=f"pf{nt}",
                              name=f"pf{nt}_{rep}") for nt in range(NT)]

                # ---- pass 1: y1 = S0 x0, F-layout [n, (b,d)] ----
                for it in range(NT):
                    isl = slice(it * 128, (it + 1) * 128)
                    ps0 = pb.tile([128, 4, 128], F32, tag="big")
                    ps1 = pb.tile([128, 4, 128], F32, tag="big")
                    for jt in range(NT):
                        lhsT = s0t[jt][:, isl]
                        nc.tensor.matmul(ps0[:], lhsT, xa[jt][:, 0:512],
                                         start=(jt == 0), stop=(jt == NT - 1))
                        nc.tensor.matmul(ps1[:], lhsT, xa[jt][:, 512:1024],
                                         start=(jt == 0), stop=(jt == NT - 1))
                    for f, ps in ((0, ps0), (1, ps1)):
                        bsl = slice(4 * f, 4 * f + 4)
                        pcopy(yb16[it][:, bsl, :], ps[:])
                        for b in range(4 * f, 4 * f + 4):
                            pst = psm.tile([128, 128], BF16, tag="tr")
                            nc.tensor.transpose(pst[:], yb16[it][:, b, :],
                                                identr[:])
                            pcopy(y1tt[b][it][:], pst[:])

                # prefetch next rep's x0f right after pass1 releases xa:
                # double-buffered, and the SP queue has no pending output
                # stores (those go on the ACT HWDGE queue) so the reload
                # overlaps pass2/P/final of this rep.
                if rep + 1 < reps:
                    xa = []
                    for j in range(NT):
                        t = mp.tile([128, BL * D], BF16, tag=f"xa{j}",
                                    name=f"xa{j}_{rep + 1}", bufs=2)
                        nc.sync.dma_start(
                            out=t[:], in_=x0f_d[j * 128:(j + 1) * 128, :])
                        xa.append(t)

                # ---- pass 2: y3T = (S1 y1)^T, T-layout [(b,d), n] ----
                for b in range(BL):
                    ps0 = pb.tile([128, 512], F32, tag="big")
                    ps1 = pb.tile([128, 512], F32, tag="big")
                    for jt in range(NT):
                        lhsT = yb16[jt][:, b, :]
                        nc.tensor.matmul(ps0[:], lhsT, s1t[jt][:, 0:512],
                                         start=(jt == 0), stop=(jt == NT - 1))
                        nc.tensor.matmul(ps1[:], lhsT, s1t[jt][:, 512:1024],
                                         start=(jt == 0), stop=(jt == NT - 1))
                    pcopy(y3t[b][:, 0:512], ps0[:])
                    pcopy(y3t[b][:, 512:1024], ps1[:])

                # ---- feature projections (bundled moving weights) ----
                # ps[:,0,:] = P1_b = x0 Wb + y1 Wc ; ps[:,1,:] = P2_b =
                # y1 Wd + y3 We.  The y1 matmul streams [Wc|Wd] as one
                # 256-wide moving operand spanning both psum rows.
                for nt in range(NT):
                    nsl = slice(nt * 128, (nt + 1) * 128)
                    for b in range(BL):
                        ps = pb.tile([128, 4, 128], F32, tag="big")
                        nc.tensor.matmul(ps[:, 0, :], x0t[b][:, nsl],
                                         w_t[:, 128:256], start=True,
                                         stop=False, skip_group_check=True)
                        nc.tensor.matmul(ps[:, 0:2, :], y1tt[b][nt][:],
                                         w_t[:, 256:512], start=False,
                                         stop=False, skip_group_check=True)
                        nc.tensor.matmul(ps[:, 1, :], y3t[b][:, nsl],
                                         w_t[:, 512:640], start=False,
                                         stop=True, skip_group_check=True)
                        pcopy(pf[nt][:, :, b * 128:(b + 1) * 128],
                              ps[:, 0:2, :])

                # ---- final: out = x0 Wa + S0 P1 + S1 P2 + bias ----
                for it in range(NT):
                    isl = slice(it * 128, (it + 1) * 128)
                    psA = pb.tile([128, 512], F32, tag="big",
                                  name=f"finA_{rep}_{it}")
                    psB = pb.tile([128, 512], F32, tag="big",
                                  name=f"finB_{rep}_{it}")
                    for jt in range(NT):
                        lhsT = s0t[jt][:, isl]
                        nc.tensor.matmul(psA[:], lhsT, pf[jt][:, 0, 0:512],
                                         start=(jt == 0), stop=False,
                                         skip_group_check=True)
                        nc.tensor.matmul(psB[:], lhsT, pf[jt][:, 0, 512:1024],
                                         start=(jt == 0), stop=False,
                                         skip_group_check=True)
                    for jt in range(NT):
                        lhsT = s1t[jt][:, isl]
                        nc.tensor.matmul(psA[:], lhsT, pf[jt][:, 1, 0:512],
                                         start=False, stop=False,
                                         skip_group_check=True)
                        nc.tensor.matmul(psB[:], lhsT, pf[jt][:, 1, 512:1024],
                                         start=False, stop=False,
                                         skip_group_check=True)
                    # x0*Wa region-adds LAST, start=False: no bank clear.
                    for q, b in enumerate(range(0, 4)):
                        nc.tensor.matmul(psA[:, q * 128:(q + 1) * 128],
                                         x0t[b][:, isl], w_t[:, 0:128],
                                         start=False, stop=(q == 3),
                                         skip_group_check=True)
                    for q, b in enumerate(range(4, 8)):
                        nc.tensor.matmul(psB[:, q * 128:(q + 1) * 128],
                                         x0t[b][:, isl], w_t[:, 0:128],
                                         start=False, stop=(q == 3),
                                         skip_group_check=True)
                    for f, ps in ((0, psA), (1, psB)):
                        ot = op.tile([128, 512], BF16, tag="out")
                        nc.vector.tensor_add(ot[:], ps[:], bias_t[:])
                        # output stores ride the ACT HWDGE queue so they never
                        # head-of-line block the SP queue's input loads
                        nc.scalar.dma_start(
                            out=out_d[isl, 4 * f:4 * f + 4, :], in_=ot[:])
    nc.compile()
    return nc


def _prep_inputs(supports, inputs, state, weight, biases):
    supports = np.asarray(supports, dtype=np.float32)
    inputs = np.asarray(inputs, dtype=np.float32)
    state = np.asarray(state, dtype=np.float32)
    weight = np.asarray(weight, dtype=np.float32)
    biases = np.asarray(biases, dtype=np.float32)

    s0t = np.ascontiguousarray(supports[0].T).astype(NP_BF16)
    s1t = np.ascontiguousarray(supports[1].T).astype(NP_BF16)

    x0 = np.concatenate(
        [inputs.reshape(B, N, D // 2), state.reshape(B, N, D // 2)], axis=2)
    x0f = np.ascontiguousarray(x0.transpose(1, 0, 2))      # [N, B, D]
    x0t = np.ascontiguousarray(x0.transpose(0, 2, 1))      # [B, D, N]
    x0t_bf = x0t.astype(NP_BF16)

    W = weight.reshape(5, D, O)
    wf = np.concatenate([
        W[0] - W[2],                # Wa
        W[1] - W[4],                # Wb
        2.0 * W[2],                 # Wc
        W[3],                       # Wd
        2.0 * W[4],                 # We
    ], axis=1).astype(NP_BF16)

    biasb = np.ascontiguousarray(np.tile(biases[None, :], (128, 4)))

    in_maps = []
    for c in range(N_CORES):
        bsl = slice(c * BL, (c + 1) * BL)
        in_maps.append({
            "s0t": s0t,
            "s1t": s1t,
            "x0f": np.ascontiguousarray(x0f[:, bsl, :]).reshape(
                N, BL * D).astype(NP_BF16),
            "x0t": np.ascontiguousarray(x0t_bf[bsl]).reshape(BL * D, N),
            "wf": wf,
            "biasb": biasb,
        })
    return in_maps


def _get_runner(reps=1):
    """Build the jitted SPMD executor once (mirrors
    bass2jax.run_bass_via_pjrt) so repeated calls don't re-trace."""
    if ("runner", reps) in _CACHE:
        return _CACHE[("runner", reps)]
    import jax
    from jax.sharding import Mesh, PartitionSpec, NamedSharding
    from concourse import bass2jax
    import concourse.mybir as mb

    try:
        jax.config.update("jax_compilation_cache_dir", "/tmp/jax_cache")
        jax.config.update("jax_persistent_cache_min_compile_time_secs", 1.0)
    except Exception:
        pass

    if ("nc", reps) not in _CACHE:
        _CACHE[("nc", reps)] = _build(reps=reps)
    nc = _CACHE[("nc", reps)]
    bass2jax.install_neuronx_cc_hook()

    part_name = nc.partition_id_tensor.name if nc.partition_id_tensor else None
    in_names, out_names, out_avals, zero_outs = [], [], [], []
    for alloc in nc.m.functions[0].allocations:
        if not isinstance(alloc, mb.MemoryLocationSet):
            continue
        name = alloc.memorylocations[0].name
        if alloc.kind == "ExternalInput":
            if name != part_name:
                in_names.append(name)
        elif alloc.kind == "ExternalOutput":
            out_names.append(name)
            shape = tuple(alloc.tensor_shape)
            dtype = mb.dt.np(alloc.dtype)
            out_avals.append(jax.core.ShapedArray(shape, dtype))
            zero_outs.append(np.zeros(shape, dtype))
    n_params = len(in_names)
    all_names = in_names + out_names
    if part_name is not None:
        all_names = all_names + [part_name]

    def _body(*args):
        operands = list(args)
        if part_name is not None:
            operands.append(bass2jax.partition_id_tensor())
        outs = bass2jax._bass_exec_p.bind(
            *operands,
            out_avals=tuple(out_avals),
            in_names=tuple(all_names),
            out_names=tuple(out_names),
            lowering_input_output_aliases=(),
            sim_require_finite=True,
            sim_require_nnan=True,
            nc=nc,
        )
        return tuple(outs)

    devices = jax.devices()[:N_CORES]
    mesh = Mesh(np.asarray(devices), ("core",))
    from jax.experimental.shard_map import shard_map
    n_outs = len(out_names)
    donate = tuple(range(n_params, n_params + n_outs))
    sharded = jax.jit(
        shard_map(_body, mesh=mesh,
                  in_specs=(PartitionSpec("core"),) * (n_params + n_outs),
                  out_specs=(PartitionSpec("core"),) * n_outs,
                  check_rep=False),
        donate_argnums=donate, keep_unused=True)
    sh = NamedSharding(mesh, PartitionSpec("core"))

    runner = {
        "fn": sharded, "in_names": in_names, "out_names": out_names,
        "zero_outs": zero_outs, "sharding": sh, "mesh": mesh,
    }
    _CACHE[("runner", reps)] = runner
    return runner


def _run(in_maps, device_inputs=None, reps=1):
    """Execute on the 8 cores; returns list of per-core output dicts."""
    import jax
    r = _get_runner(reps)
    if device_inputs is None:
        device_inputs = _put_inputs(in_maps, reps)
    zeros = [
        jax.device_put(
            np.zeros((N_CORES * z.shape[0], *z.shape[1:]), z.dtype),
            r["sharding"])
        for z in r["zero_outs"]
    ]
    out_arrs = r["fn"](*device_inputs, *zeros)
    outs = [np.asarray(a) for a in out_arrs]
    return [
        {name: outs[i].reshape(N_CORES, *r["zero_outs"][i].shape)[c]
         for i, name in enumerate(r["out_names"])}
        for c in range(N_CORES)
    ]


def _put_inputs(in_maps, reps=1):
    import jax
    r = _get_runner(reps)
    return [
        jax.device_put(
            np.concatenate([np.asarray(in_maps[c][n]) for c in range(N_CORES)],
                           axis=0), r["sharding"])
        for n in r["in_names"]
    ]


def kernel(supports, inputs, state, weight, biases, output_size=O, **_):
    assert int(output_size) == O
    in_maps = _prep_inputs(supports, inputs, state, weight, biases)
    res = _run(in_maps)
    # per-core out: [N, BL, O] -> full [B, N*O]
    outs = np.stack([np.asarray(res[c]["out"], dtype=np.float32)
                     for c in range(N_CORES)])
    out = outs.transpose(0, 2, 1, 3).reshape(B, N * O)
    return np.ascontiguousarray(out)


if __name__ == "__main__":
    rng = np.random.default_rng(0)
    sup = rng.standard_normal((2, N, N)).astype(np.float32) / np.sqrt(N)
    inp = rng.standard_normal((B, N * 64)).astype(np.float32)
    st = rng.standard_normal((B, N * 64)).astype(np.float32)
    wt = rng.standard_normal((5 * D, O)).astype(np.float32) * 0.05
    bs = np.zeros((O,), np.float32)
    out = kernel(sup, inp, st, wt, bs, O)
    print("out", out.shape, out.dtype, float(np.abs(out).max()))


# revision 37
# speedup vs baseline: 1.0689x; 1.0689x over previous
"""Trainium2 Bass kernel for nn_DiffusionGraphConv (gnn_message_passing).

Reference computation (B=64, N=1024, D=128=64+64, O=128, 2 supports,
2 diffusion steps):
    x0 = concat(inputs, state)                      # [B, N, D]
    y1 = S0 x0 ; z2 = S0 y1 ; y3 = S1 y1 ; z4 = S1 y3
    xs = [x0, y1, 2 z2 - x0, y3, 2 z4 - y1]
    out = concat_d(xs) @ W + bias                   # [B*N, O]

Algebraic refactor (host folds the +-/2x into the weight blocks, and the
feature projection commutes with the node-space supports):
    Wa = W0 - W2, Wb = W1 - W4, Wc = 2 W2, Wd = W3, We = 2 W4
    out = x0 Wa + S0 (x0 Wb + y1 Wc) + S1 (y1 Wd + y3 We)

Sharding: data-parallel over batch, 8 batches per NeuronCore, supports and
weights replicated. Per-core schedule (f32 PSUM accumulation, bf16 MMs):
    pass1: y1   = S0 x0          F-layout [n, (b,d)], f-paired FD512 MMs
    trans: y1T  via DMA XBAR     (off the PE; dedicated [128,128] dst tiles)
    pass2: y3T  = (S1 y1)^T      f-paired FD512 bf16 MMs
    feat:  P1 = x0 Wb + y1 Wc ; P2 = y1 Wd + y3 We   (bundled [Wc|Wd]
           moving operand; one strided copy into the merged pf tile)
    final: out = x0 Wa + S0 P1 + S1 P2 + bias        (FD512 MMs + regions)
Per-rep reloaded tensors (xa) do not alias late-phase consumers, so the
next rep's input DMA overlaps the current rep's compute.
"""
import sys

if "/opt/trn_rl_repo" not in sys.path:
    sys.path.insert(0, "/opt/trn_rl_repo")

import numpy as np
import ml_dtypes

import concourse.bass as bass
import concourse.mybir as mybir
from concourse import bacc, tile
from concourse.bass_utils import run_bass_kernel_spmd
from concourse.masks import make_identity

N_CORES = 8
B = 64
BL = B // N_CORES          # local batches per core
N = 1024                   # nodes
D = 128                    # input_size (64 input + 64 hidden)
O = 128                    # output_size
NT = N // 128              # node partition tiles
F32 = mybir.dt.float32
BF16 = mybir.dt.bfloat16
FP8 = mybir.dt.float8e4
DR = mybir.MatmulPerfMode.DoubleRow
NP_FP8 = ml_dtypes.float8_e4m3
NP_BF16 = ml_dtypes.bfloat16

_CACHE = {}


def _build(reps=1):
    nc = bacc.Bacc("TRN2", target_bir_lowering=False, debug=False,
                   num_devices=N_CORES)
    s0t_d = nc.dram_tensor("s0t", [N, N], BF16, kind="ExternalInput").ap()
    s1t_d = nc.dram_tensor("s1t", [N, N], BF16, kind="ExternalInput").ap()
    x0f_d = nc.dram_tensor("x0f", [N, BL * D], BF16, kind="ExternalInput").ap()
    x0t_d = nc.dram_tensor("x0t", [BL * D, N], BF16, kind="ExternalInput").ap()
    # weights side by side: [Wa | Wb | Wc | Wd | We]  ([Wc|Wd] is one
    # contiguous 256-wide moving operand for the bundled y1 projection)
    wf_d = nc.dram_tensor("wf", [D, 5 * O], BF16, kind="ExternalInput").ap()
    bias_d = nc.dram_tensor("biasb", [128, 512], F32, kind="ExternalInput").ap()
    out_d = nc.dram_tensor("out", [N, BL, O], BF16, kind="ExternalOutput").ap()

    with tile.TileContext(nc) as tc:
        with (
            tc.tile_pool(name="main", bufs=1) as mp,
            tc.tile_pool(name="outp", bufs=4) as op,
            tc.tile_pool(name="psb", bufs=5, space="PSUM") as pb,
            tc.tile_pool(name="dstage", bufs=2, space="DRAM") as dp,
        ):
            # ---- persistent SBUF residents (DMA in consumption order) ----
            s0t = []
            xa0 = []
            for j in range(NT):
                t = mp.tile([128, N], BF16, tag=f"s0t{j}", name=f"s0t{j}")
                nc.sync.dma_start(out=t[:], in_=s0t_d[j * 128:(j + 1) * 128, :])
                s0t.append(t)
                t = mp.tile([128, BL * D], BF16, tag=f"xa{j}", name=f"xa{j}_p")
                nc.sync.dma_start(out=t[:], in_=x0f_d[j * 128:(j + 1) * 128, :])
                xa0.append(t)
            s1t = []
            for j in range(NT):
                t = mp.tile([128, N], BF16, tag=f"s1t{j}", name=f"s1t{j}")
                nc.sync.dma_start(out=t[:], in_=s1t_d[j * 128:(j + 1) * 128, :])
                s1t.append(t)
            x0t = []
            for b in range(BL):
                t = mp.tile([128, N], BF16, tag=f"x0t{b}", name=f"x0t{b}")
                nc.sync.dma_start(out=t[:], in_=x0t_d[b * 128:(b + 1) * 128, :])
                x0t.append(t)
            w_t = mp.tile([128, 5 * O], BF16, tag="wt")
            nc.sync.dma_start(out=w_t[:], in_=wf_d[:])
            bias_t = mp.tile([128, 512], F32, tag="bias")
            nc.sync.dma_start(out=bias_t[:], in_=bias_d[:])

            ci = 0

            def pcopy(dst, src):
                # alternate DVE / ACT for PSUM->SBUF moves
                nonlocal ci
                if ci % 2 == 0:
                    nc.vector.tensor_copy(dst, src)
                else:
                    nc.scalar.copy(dst, src)
                ci += 1

            def pscale(dst, src, mul):
                nonlocal ci
                if ci % 2 == 0:
                    nc.vector.tensor_scalar_mul(dst, src, mul)
                else:
                    nc.scalar.mul(dst, src, mul)
                ci += 1

            xa = xa0
            for rep in range(reps):
                # per-rep buffers
                yb16 = [mp.tile([128, BL, D], BF16, tag=f"yb{it}",
                                name=f"yb{it}_{rep}") for it in range(NT)]
                ybd = dp.tile([N, BL, D], BF16, tag="ybd")
                y1t = [mp.tile([128, N], BF16, tag=f"y1t{b}",
                               name=f"y1t{b}_{rep}") for b in range(BL)]
                y3t = [mp.tile([128, N], BF16, tag=f"y3t{b}",
                               name=f"y3t{b}_{rep}") for b in range(BL)]
                # merged projections: pf[nt][:, 0, :] = P1, [:, 1, :] = P2
                pf = [mp.tile([128, 2, BL * O], BF16, tag=f"pf{nt}",
                              name=f"pf{nt}_{rep}") for nt in range(NT)]
                # x0 Wa + bias, f32 (final adds it straight onto the psum)
                base = [mp.tile([128, BL * O], F32, tag=f"base{nt}",
                                name=f"base{nt}_{rep}") for nt in range(NT)]

                # ---- pass 1: y1 = S0 x0, F-layout [n, (b,d)] ----
                for it in range(NT):
                    isl = slice(it * 128, (it + 1) * 128)
                    ps0 = pb.tile([128, 4, 128], F32, tag="big")
                    ps1 = pb.tile([128, 4, 128], F32, tag="big")
                    for jt in range(NT):
                        lhsT = s0t[jt][:, isl]
                        nc.tensor.matmul(ps0[:], lhsT, xa[jt][:, 0:512],
                                         start=(jt == 0), stop=(jt == NT - 1))
                        nc.tensor.matmul(ps1[:], lhsT, xa[jt][:, 512:1024],
                                         start=(jt == 0), stop=(jt == NT - 1))
                    for f, ps in ((0, ps0), (1, ps1)):
                        bsl = slice(4 * f, 4 * f + 4)
                        pcopy(yb16[it][:, bsl, :], ps[:])
                        # stage y1 to DRAM for the XBAR transposes (ACT queue)
                        nc.scalar.dma_start(out=ybd[isl, bsl, :],
                                            in_=yb16[it][:, bsl, :])

                # y1T via DMA XBAR transposes, DRAM -> SBUF: one [N,128] ->
                # [128,N] transpose per batch, off the PE entirely.  The dram
                # pool tile carries the store->transpose dependency.
                for b in range(BL):
                    nc.scalar.dma_start(out=y1t[b][:], in_=ybd[:, b, :],
                                        transpose=True)

                # prefetch next rep's x0f right after pass1 releases xa:
                # double-buffered, and the SP queue has no pending output
                # stores (those go on the ACT HWDGE queue) so the reload
                # overlaps pass2/P/final of this rep.
                if rep + 1 < reps:
                    xa = []
                    for j in range(NT):
                        t = mp.tile([128, BL * D], BF16, tag=f"xa{j}",
                                    name=f"xa{j}_{rep + 1}")
                        nc.sync.dma_start(
                            out=t[:], in_=x0f_d[j * 128:(j + 1) * 128, :])
                        xa.append(t)

                # ---- pass 2: y3T = (S1 y1)^T, T-layout [(b,d), n] ----
                for b in range(BL):
                    ps0 = pb.tile([128, 512], F32, tag="big")
                    ps1 = pb.tile([128, 512], F32, tag="big")
                    for jt in range(NT):
                        lhsT = yb16[jt][:, b, :]
                        nc.tensor.matmul(ps0[:], lhsT, s1t[jt][:, 0:512],
                                         start=(jt == 0), stop=(jt == NT - 1))
                        nc.tensor.matmul(ps1[:], lhsT, s1t[jt][:, 512:1024],
                                         start=(jt == 0), stop=(jt == NT - 1))
                    pcopy(y3t[b][:, 0:512], ps0[:])
                    pcopy(y3t[b][:, 512:1024], ps1[:])

                # ---- feature projections (bundled moving weights) ----
                # ps rows: 0 = x0 Wa (-> base), 1 = P1 = x0 Wb + y1 Wc,
                # 2 = P2 = y1 Wd + y3 We.  x0 streams [Wa|Wb] and y1
                # streams [Wc|Wd], each one 256-wide moving operand.
                # b-outer so each batch only needs its own y1 transpose.
                for b in range(BL):
                    bq = (b % 4) * 128
                    for nt in range(NT):
                        nsl = slice(nt * 128, (nt + 1) * 128)
                        ps = pb.tile([128, 4, 128], F32, tag="big")
                        nc.tensor.matmul(ps[:, 0:2, :], x0t[b][:, nsl],
                                         w_t[:, 0:256], start=True,
                                         stop=False, skip_group_check=True)
                        nc.tensor.matmul(ps[:, 1:3, :], y1t[b][:, nsl],
                                         w_t[:, 256:512], start=False,
                                         stop=False, skip_group_check=True)
                        nc.tensor.matmul(ps[:, 2, :], y3t[b][:, nsl],
                                         w_t[:, 512:640], start=False,
                                         stop=True, skip_group_check=True)
                        # fold bias in while draining the Wa row (DVE)
                        nc.vector.tensor_add(base[nt][:, b * 128:(b + 1) * 128],
                                             ps[:, 0, :],
                                             bias_t[:, bq:bq + 128])
                        nc.scalar.copy(pf[nt][:, :, b * 128:(b + 1) * 128],
                                       ps[:, 1:3, :])

                # ---- final: out = x0 Wa + S0 P1 + S1 P2 + bias ----
                for it in range(NT):
                    isl = slice(it * 128, (it + 1) * 128)
                    psA = pb.tile([128, 512], F32, tag="big",
                                  name=f"finA_{rep}_{it}")
                    psB = pb.tile([128, 512], F32, tag="big",
                                  name=f"finB_{rep}_{it}")
                    for jt in range(NT):
                        lhsT = s0t[jt][:, isl]
                        nc.tensor.matmul(psA[:], lhsT, pf[jt][:, 0, 0:512],
                                         start=(jt == 0), stop=False,
                                         skip_group_check=True)
                        nc.tensor.matmul(psB[:], lhsT, pf[jt][:, 0, 512:1024],
                                         start=(jt == 0), stop=False,
                                         skip_group_check=True)
                    for jt in range(NT):
                        lhsT = s1t[jt][:, isl]
                        nc.tensor.matmul(psA[:], lhsT, pf[jt][:, 1, 0:512],
                                         start=False, stop=(jt == NT - 1),
                                         skip_group_check=True)
                        nc.tensor.matmul(psB[:], lhsT, pf[jt][:, 1, 512:1024],
                                         start=False, stop=(jt == NT - 1),
                                         skip_group_check=True)
                    for f, ps in ((0, psA), (1, psB)):
                        ot = op.tile([128, 512], BF16, tag="out")
                        # base carries x0 Wa + bias (f32)
                        nc.vector.tensor_add(
                            ot[:], ps[:], base[it][:, f * 512:(f + 1) * 512])
                        # output stores ride the ACT HWDGE queue so they never
                        # head-of-line block the SP queue's input loads
                        nc.scalar.dma_start(
                            out=out_d[isl, 4 * f:4 * f + 4, :], in_=ot[:])
    nc.compile()
    return nc


def _prep_inputs(supports, inputs, state, weight, biases):
    supports = np.asarray(supports, dtype=np.float32)
    inputs = np.asarray(inputs, dtype=np.float32)
    state = np.asarray(state, dtype=np.float32)
    weight = np.asarray(weight, dtype=np.float32)
    biases = np.asarray(biases, dtype=np.float32)

    s0t = np.ascontiguousarray(supports[0].T).astype(NP_BF16)
    s1t = np.ascontiguousarray(supports[1].T).astype(NP_BF16)

    x0 = np.concatenate(
        [inputs.reshape(B, N, D // 2), state.reshape(B, N, D // 2)], axis=2)
    x0f = np.ascontiguousarray(x0.transpose(1, 0, 2))      # [N, B, D]
    x0t = np.ascontiguousarray(x0.transpose(0, 2, 1))      # [B, D, N]
    x0t_bf = x0t.astype(NP_BF16)

    W = weight.reshape(5, D, O)
    wf = np.concatenate([
        W[0] - W[2],                # Wa
        W[1] - W[4],                # Wb
        2.0 * W[2],                 # Wc
        W[3],                       # Wd
        2.0 * W[4],                 # We
    ], axis=1).astype(NP_BF16)

    biasb = np.ascontiguousarray(np.tile(biases[None, :], (128, 4)))

    in_maps = []
    for c in range(N_CORES):
        bsl = slice(c * BL, (c + 1) * BL)
        in_maps.append({
            "s0t": s0t,
            "s1t": s1t,
            "x0f": np.ascontiguousarray(x0f[:, bsl, :]).reshape(
                N, BL * D).astype(NP_BF16),
            "x0t": np.ascontiguousarray(x0t_bf[bsl]).reshape(BL * D, N),
            "wf": wf,
            "biasb": biasb,
        })
    return in_maps


def _get_runner(reps=1):
    """Build the jitted SPMD executor once (mirrors
    bass2jax.run_bass_via_pjrt) so repeated calls don't re-trace."""
    if ("runner", reps) in _CACHE:
        return _CACHE[("runner", reps)]
    import jax
    from jax.sharding import Mesh, PartitionSpec, NamedSharding
    from concourse import bass2jax
    import concourse.mybir as mb

    try:
        jax.config.update("jax_compilation_cache_dir", "/tmp/jax_cache")
        jax.config.update("jax_persistent_cache_min_compile_time_secs", 1.0)
    except Exception:
        pass

    if ("nc", reps) not in _CACHE:
        _CACHE[("nc", reps)] = _build(reps=reps)
    nc = _CACHE[("nc", reps)]
    bass2jax.install_neuronx_cc_hook()

    part_name = nc.partition_id_tensor.name if nc.partition_id_tensor else None
    in_names, out_names, out_avals, zero_outs = [], [], [], []
    for alloc in nc.m.functions[0].allocations:
        if not isinstance(alloc, mb.MemoryLocationSet):
            continue
        name = alloc.memorylocations[0].name
        if alloc.kind == "ExternalInput":
            if name != part_name:
                in_names.append(name)
        elif alloc.kind == "ExternalOutput":
            out_names.append(name)
            shape = tuple(alloc.tensor_shape)
            dtype = mb.dt.np(alloc.dtype)
            out_avals.append(jax.core.ShapedArray(shape, dtype))
            zero_outs.append(np.zeros(shape, dtype))
    n_params = len(in_names)
    all_names = in_names + out_names
    if part_name is not None:
        all_names = all_names + [part_name]

    def _body(*args):
        operands = list(args)
        if part_name is not None:
            operands.append(bass2jax.partition_id_tensor())
        outs = bass2jax._bass_exec_p.bind(
            *operands,
            out_avals=tuple(out_avals),
            in_names=tuple(all_names),
            out_names=tuple(out_names),
            lowering_input_output_aliases=(),
            sim_require_finite=True,
            sim_require_nnan=True,
            nc=nc,
        )
        return tuple(outs)

    devices = jax.devices()[:N_CORES]
    mesh = Mesh(np.asarray(devices), ("core",))
    from jax.experimental.shard_map import shard_map
    n_outs = len(out_names)
    donate = tuple(range(n_params, n_params + n_outs))
    sharded = jax.jit(
        shard_map(_body, mesh=mesh,
                  in_specs=(PartitionSpec("core"),) * (n_params + n_outs),
                  out_specs=(PartitionSpec("core"),) * n_outs,
                  check_rep=False),
        donate_argnums=donate, keep_unused=True)
    sh = NamedSharding(mesh, PartitionSpec("core"))

    runner = {
        "fn": sharded, "in_names": in_names, "out_names": out_names,
        "zero_outs": zero_outs, "sharding": sh, "mesh": mesh,
    }
    _CACHE[("runner", reps)] = runner
    return runner


def _run(in_maps, device_inputs=None, reps=1):
    """Execute on the 8 cores; returns list of per-core output dicts."""
    import jax
    r = _get_runner(reps)
    if device_inputs is None:
        device_inputs = _put_inputs(in_maps, reps)
    zeros = [
        jax.device_put(
            np.zeros((N_CORES * z.shape[0], *z.shape[1:]), z.dtype),
            r["sharding"])
        for z in r["zero_outs"]
    ]
    out_arrs = r["fn"](*device_inputs, *zeros)
    outs = [np.asarray(a) for a in out_arrs]
    return [
        {name: outs[i].reshape(N_CORES, *r["zero_outs"][i].shape)[c]
         for i, name in enumerate(r["out_names"])}
        for c in range(N_CORES)
    ]


def _put_inputs(in_maps, reps=1):
    import jax
    r = _get_runner(reps)
    return [
        jax.device_put(
            np.concatenate([np.asarray(in_maps[c][n]) for c in range(N_CORES)],
                           axis=0), r["sharding"])
        for n in r["in_names"]
    ]


def kernel(supports, inputs, state, weight, biases, output_size=O, **_):
    assert int(output_size) == O
    in_maps = _prep_inputs(supports, inputs, state, weight, biases)
    res = _run(in_maps)
    # per-core out: [N, BL, O] -> full [B, N*O]
    outs = np.stack([np.asarray(res[c]["out"], dtype=np.float32)
                     for c in range(N_CORES)])
    out = outs.transpose(0, 2, 1, 3).reshape(B, N * O)
    return np.ascontiguousarray(out)


if __name__ == "__main__":
    rng = np.random.default_rng(0)
    sup = rng.standard_normal((2, N, N)).astype(np.float32) / np.sqrt(N)
    inp = rng.standard_normal((B, N * 64)).astype(np.float32)
    st = rng.standard_normal((B, N * 64)).astype(np.float32)
    wt = rng.standard_normal((5 * D, O)).astype(np.float32) * 0.05
    bs = np.zeros((O,), np.float32)
    out = kernel(sup, inp, st, wt, bs, O)
    print("out", out.shape, out.dtype, float(np.abs(out).max()))
